# revision 1
# baseline (speedup 1.0000x reference)
"""Trainium2 Bass kernel for the Bengio03 Highway BiLM problem.

Math (see reference): L=3 layers, each with fwd/bwd chains. Per (layer, dir):
  padded = [front_pads(4), seq(512), back_pads(4)]          # [B, 520, H]
  pre[t] = sum_{k=0..4} padded[t + k + off] @ W[k*H:(k+1)*H]  (off=0 fwd, 4 bwd)
  x0 = relu(pre + b)
  2x highway: proj = x @ Ws[j] + bs[j]; nonlin,gate = split(proj)
              x = sigmoid(gate)*x + (1-sigmoid(gate))*relu(nonlin)
  out[l,:,:, 0:512] = f chain, [512:1024] = b chain

Implementation strategy (per core, data-parallel over batch: 4 seqs/core):
  - Activations kept feature-major in SBUF: xT tiles [128(h), 520(t)] per
    (h_chunk, b). Matmuls compute outT = W_tile.T @ xT directly (weights are
    the stationary lhsT, activations stream as rhs with N=512 tokens), so no
    transposes are needed between layers. The 5-tap conv is 5 shifted rhs
    slices accumulated in PSUM (20 matmuls of [128,128]@[128,512] per output
    chunk).
  - fp32r (FP22-truncated fp32) matmuls: full PE rate, ~2^-14 relative error.
  - PE transposes only at the edges: input load (token-major -> feature-major)
    and output store.
  - Elementwise: ScalarE does relu/sigmoid (+bias, PSUM->SBUF), VectorE does
    the 3 tensor_tensor ops of the highway combine, GpSimd writes pad columns.
"""

import os
import sys

sys.path.insert(0, "/opt/trn_rl_repo")

import numpy as np

import concourse.bass as bass
import concourse.bacc as bacc
import concourse.tile as tile
from concourse import mybir
from concourse.bass_utils import run_bass_kernel_spmd

# Problem constants (hardcoded per spec).
L = 3
WIDTH = 4
H = 512
B = 32
S = 512
NHW = 2
CIN = (WIDTH + 1) * H  # 2560
NCORES = 8
BLOC = B // NCORES  # 4 sequences per core
HC = H // 128  # 4 hidden chunks of 128
SPAD = S + 2 * WIDTH  # 520
NT = S // 128  # 4 token tiles of 128
F32 = mybir.dt.float32
F32R = mybir.dt.float32r
RELU = mybir.ActivationFunctionType.Relu
SIGM = mybir.ActivationFunctionType.Sigmoid
ADD = mybir.AluOpType.add
MAX = mybir.AluOpType.max


def _build_program():
    nc = bacc.Bacc(
        "TRN2",
        target_bir_lowering=False,
        debug=False,
        enable_asserts=False,
        num_devices=1,
    )

    x_d = nc.dram_tensor("x", [BLOC, S, H], F32, kind="ExternalInput").ap()
    fw_d = nc.dram_tensor("fw", [L, CIN, H], F32, kind="ExternalInput").ap()
    bw_d = nc.dram_tensor("bw", [L, CIN, H], F32, kind="ExternalInput").ap()
    fhw_d = nc.dram_tensor("fhw", [L, NHW, H, 2 * H], F32, kind="ExternalInput").ap()
    bhw_d = nc.dram_tensor("bhw", [L, NHW, H, 2 * H], F32, kind="ExternalInput").ap()
    fbt_d = nc.dram_tensor("fbt", [L, 128, HC], F32, kind="ExternalInput").ap()
    bbt_d = nc.dram_tensor("bbt", [L, 128, HC], F32, kind="ExternalInput").ap()
    fhbt_d = nc.dram_tensor("fhbt", [L, NHW, 128, 2 * HC], F32, kind="ExternalInput").ap()
    bhbt_d = nc.dram_tensor("bhbt", [L, NHW, 128, 2 * HC], F32, kind="ExternalInput").ap()
    padf_d = nc.dram_tensor("padf", [L, H, WIDTH], F32, kind="ExternalInput").ap()
    padb_d = nc.dram_tensor("padb", [L, H, WIDTH], F32, kind="ExternalInput").ap()
    id_d = nc.dram_tensor("ident", [128, 128], F32, kind="ExternalInput").ap()
    out_d = nc.dram_tensor("out", [L, BLOC, S, 2 * H], F32, kind="ExternalOutput").ap()

    with tile.TileContext(nc) as tc:
        with (
            tc.tile_pool(name="consts", bufs=1) as consts,
            tc.tile_pool(name="acts", bufs=32) as acts,
            tc.tile_pool(name="convw", bufs=20) as convw,
            tc.tile_pool(name="hww", bufs=8) as hww,
            tc.tile_pool(name="xmid", bufs=8) as xmid,
            tc.tile_pool(name="work", bufs=3) as work,
            tc.tile_pool(name="stg", bufs=3) as stg,
            tc.tile_pool(name="psum", bufs=2, space="PSUM") as psum,
        ):
            # ---- constants ----
            ident = consts.tile([128, 128], F32, name="identc", tag="ident", bufs=1)
            nc.sync.dma_start(ident[:], id_d[:, :])

            pads = {}
            for l in range(L):
                for c in range(HC):
                    pt = consts.tile(
                        [128, 2 * WIDTH], F32, name=f"pad_{l}_{c}", tag="pads",
                        bufs=L * HC,
                    )
                    nc.sync.dma_start(pt[:, 0:WIDTH], padf_d[l, c * 128:(c + 1) * 128, :])
                    nc.sync.dma_start(pt[:, WIDTH:], padb_d[l, c * 128:(c + 1) * 128, :])
                    pads[(l, c)] = pt

            cbias = {}
            for dirc, src in (("f", fbt_d), ("b", bbt_d)):
                for l in range(L):
                    t = consts.tile([128, HC], F32, name=f"cb_{dirc}{l}", tag="cb", bufs=2 * L)
                    nc.sync.dma_start(t[:], src[l])
                    cbias[(dirc, l)] = t
            hbias = {}
            for dirc, src in (("f", fhbt_d), ("b", bhbt_d)):
                for l in range(L):
                    for j in range(NHW):
                        t = consts.tile(
                            [128, 2 * HC], F32, name=f"hb_{dirc}{l}_{j}", tag="hb",
                            bufs=2 * L * NHW,
                        )
                        nc.sync.dma_start(t[:], src[l, j])
                        hbias[(dirc, l, j)] = t

            def new_act_tile(name):
                return acts.tile([128, SPAD], F32, name=name, tag="acts", bufs=32)

            def write_pads(at, l, c):
                nc.gpsimd.tensor_copy(at[:, 0:WIDTH].bitcast(F32R), pads[(l, c)][:, 0:WIDTH])
                nc.gpsimd.tensor_copy(at[:, WIDTH + S:SPAD].bitcast(F32R), pads[(l, c)][:, WIDTH:])

            loaded_cw = {}
            loaded_hw = {}

            def ensure_convw(dirc, l):
                if (dirc, l) not in loaded_cw:
                    loaded_cw[(dirc, l)] = load_convw(dirc, l)
                return loaded_cw[(dirc, l)]

            def ensure_hww(dirc, l):
                if (dirc, l) not in loaded_hw:
                    loaded_hw[(dirc, l)] = load_hww(dirc, l)
                return loaded_hw[(dirc, l)]

            # ---- weight loading ----
            def load_convw(dirc, l):
                src = fw_d if dirc == "f" else bw_d
                tiles = []
                for r in range(CIN // 128):  # 20 chunks of 128 rows
                    w = convw.tile([128, H], F32, name=f"cw_{dirc}{l}_{r}", tag="convw", bufs=27)
                    nc.sync.dma_start(
                        w[:].bitcast(F32R),
                        src[l, r * 128:(r + 1) * 128, :].bitcast(F32R),
                    )
                    tiles.append(w)
                return tiles

            def load_hww(dirc, l):
                src = fhw_d if dirc == "f" else bhw_d
                res = []
                for j in range(NHW):
                    jt = []
                    for h in range(HC):
                        w = hww.tile([128, 2 * H], F32, name=f"hw_{dirc}{l}_{j}_{h}", tag="hww", bufs=8)
                        nc.scalar.dma_start(
                            w[:].bitcast(F32R),
                            src[l, j, h * 128:(h + 1) * 128, :].bitcast(F32R),
                        )
                        jt.append(w)
                    res.append(jt)
                return res

            # ---- input stage: load + transpose to feature-major ----
            # All 16 raw-tile loads dispatch first (from spare acts-pool slots)
            # so the Act engine's copies never block later loads.
            ensure_convw("f", 0)
            xT = {}
            raws = {}
            for b in range(BLOC):
                for t4 in range(NT):
                    raw = acts.tile([128, H], F32, name=f"inraw_{b}_{t4}", tag="acts", bufs=32)
                    nc.scalar.dma_start(raw[:], x_d[b, t4 * 128:(t4 + 1) * 128, :])
                    raws[(b, t4)] = raw
            for b in range(BLOC):
                for c in range(HC):
                    at = new_act_tile(f"xT_{c}_{b}")
                    tp = psum.tile([128, S], F32, name=f"tpi_{c}_{b}", tag="hpsum", bufs=4)
                    for t4 in range(NT):
                        nc.tensor.matmul(
                            tp[:, t4 * 128:(t4 + 1) * 128],
                            lhsT=raws[(b, t4)][:, c * 128:(c + 1) * 128],
                            rhs=ident[:],
                            is_transpose=True,
                        )
                    nc.scalar.copy(at[:, WIDTH:WIDTH + S].bitcast(F32R), tp[:])
                    write_pads(at, 0, c)
                    xT[(c, b)] = at

            # ---- stages ----
            def conv_stage(dirc, l, srcset, pair, wtiles):
                off0 = 0 if dirc == "f" else WIDTH
                x0 = {}
                for b in pair:
                    for n in range(HC):
                        ps = psum.tile([128, S], F32, name=f"cps_{b}_{n}", tag="cpsum", bufs=4)
                        for r in range(20):
                            k, ci = divmod(r, HC)
                            off = off0 + k
                            nc.tensor.matmul(
                                ps[:],
                                lhsT=wtiles[r][:, n * 128:(n + 1) * 128].bitcast(F32R),
                                rhs=srcset[(ci, b)][:, off:off + S].bitcast(F32R),
                                start=(r == 0),
                                stop=(r == 19),
                            )
                        xt = xmid.tile([128, S], F32, name=f"x0_{b}_{n}", tag="x0", bufs=8)
                        nc.vector.tensor_scalar(
                            xt[:].bitcast(F32R), ps[:], cbias[(dirc, l)][:, n:n + 1], 0.0, ADD, MAX
                        )
                        x0[(n, b)] = xt
                return x0

            def hw_stage(dirc, l, j, srcset, pair, wtiles, final):
                hb = hbias[(dirc, l, j)]
                outs = {}
                for b in pair:
                    for c in range(HC):
                        pnl = psum.tile([128, S], F32, name=f"hnl_{b}_{c}", tag="hpsum", bufs=4)
                        for h in range(HC):
                            nc.tensor.matmul(
                                pnl[:],
                                lhsT=wtiles[h][:, c * 128:(c + 1) * 128].bitcast(F32R),
                                rhs=srcset[(h, b)][:].bitcast(F32R),
                                start=(h == 0),
                                stop=(h == HC - 1),
                            )
                        pgt = psum.tile([128, S], F32, name=f"hgt_{b}_{c}", tag="hpsum", bufs=4)
                        for h in range(HC):
                            nc.tensor.matmul(
                                pgt[:],
                                lhsT=wtiles[h][:, H + c * 128:H + (c + 1) * 128].bitcast(F32R),
                                rhs=srcset[(h, b)][:].bitcast(F32R),
                                start=(h == 0),
                                stop=(h == HC - 1),
                            )
                        r = work.tile([128, S], F32, name=f"r_{b}_{c}", tag="r", bufs=3)
                        nc.scalar.activation(r[:], pnl[:], RELU, bias=hb[:, c:c + 1])
                        g = work.tile([128, S], F32, name=f"g_{b}_{c}", tag="g", bufs=3)
                        nc.scalar.activation(g[:], pgt[:], SIGM, bias=hb[:, HC + c:HC + c + 1])
                        d = work.tile([128, S], F32, name=f"d_{b}_{c}", tag="d", bufs=3)
                        nc.vector.tensor_sub(d[:], srcset[(c, b)][:], r[:])
                        nc.vector.tensor_mul(d[:], g[:], d[:])
                        if final:
                            at = new_act_tile(f"a_{dirc}{l}_{c}_{b}")
                            nc.vector.tensor_add(at[:, WIDTH:WIDTH + S].bitcast(F32R), d[:], r[:])
                            if l + 1 < L:
                                write_pads(at, l + 1, c)
                            emit_out(dirc, l, at, c, b)
                            outs[(c, b)] = at
                        else:
                            o = xmid.tile([128, S], F32, name=f"x1_{b}_{c}", tag="x1", bufs=8)
                            nc.vector.tensor_add(o[:].bitcast(F32R), d[:], r[:])
                            outs[(c, b)] = o
                return outs

            def emit_out(dirc, l, at, c, b):
                # 32x32 block-transpose on DVE, then block-strided DMAs whose
                # DRAM access pattern undoes the blocking: keeps the PE free.
                doff = 0 if dirc == "f" else H
                dual_queue = dirc == "b" and l == L - 1
                st = work.tile([128, S], F32, name=f"tt_{b}_{c}", tag="tt", bufs=2)
                nc.vector.transpose(st[:], at[:, WIDTH:WIDTH + S])
                for pi in range(4):
                    h0 = doff + c * 128 + 32 * pi
                    dst = out_d[l, b][:, h0:h0 + 32]
                    dst3 = dst.rearrange("(fj a) bb -> a fj bb", a=32)
                    src3 = st[32 * pi:32 * (pi + 1), :].rearrange(
                        "p (fj bb) -> p fj bb", bb=32
                    )
                    eng = nc.sync if (dual_queue and (c + pi) % 2 == 0) else nc.scalar
                    eng.dma_start(dst3, src3)

            # ---- main chain: f fully, then b (xT stays resident for b) ----
            PAIRS = [(0, 1), (2, 3)]
            for dirc in ("f", "b"):
                cur = xT
                for l in range(L):
                    cw = ensure_convw(dirc, l)
                    hw = ensure_hww(dirc, l)
                    nxt = {}
                    for pair in PAIRS:
                        x0 = conv_stage(dirc, l, cur, pair, cw)
                        x1 = hw_stage(dirc, l, 0, x0, pair, hw[0], final=False)
                        res = hw_stage(dirc, l, 1, x1, pair, hw[1], final=True)
                        nxt.update(res)
                    cur = nxt

    nc.compile()
    return nc


_CACHE = {}


def _get_program():
    if "nc" not in _CACHE:
        _CACHE["nc"] = _build_program()
    return _CACHE["nc"]


def _round_fp22(a):
    b = np.ascontiguousarray(a, dtype=np.float32).view(np.uint32)
    r = (b + np.uint32(0x1FF) + ((b >> np.uint32(10)) & np.uint32(1))) & np.uint32(0xFFFFFC00)
    return r.view(np.float32)


def _make_in_maps(inputs):
    x = np.ascontiguousarray(inputs["inputs"], dtype=np.float32)
    fw = _round_fp22(inputs["fwd_W"])
    bw = _round_fp22(inputs["bwd_W"])
    fhw = _round_fp22(inputs["fwd_hw_W"])
    bhw = _round_fp22(inputs["bwd_hw_W"])
    fbt = np.ascontiguousarray(
        np.asarray(inputs["fwd_b"], dtype=np.float32).reshape(L, HC, 128).transpose(0, 2, 1)
    )
    bbt = np.ascontiguousarray(
        np.asarray(inputs["bwd_b"], dtype=np.float32).reshape(L, HC, 128).transpose(0, 2, 1)
    )
    fhbt = np.ascontiguousarray(
        np.asarray(inputs["fwd_hw_b"], dtype=np.float32)
        .reshape(L, NHW, 2 * HC, 128)
        .transpose(0, 1, 3, 2)
    )
    bhbt = np.ascontiguousarray(
        np.asarray(inputs["bwd_hw_b"], dtype=np.float32)
        .reshape(L, NHW, 2 * HC, 128)
        .transpose(0, 1, 3, 2)
    )
    padf = np.ascontiguousarray(
        np.asarray(inputs["fwd_pads"], dtype=np.float32).transpose(0, 2, 1)
    )
    padb = np.ascontiguousarray(
        np.asarray(inputs["bwd_pads"], dtype=np.float32).transpose(0, 2, 1)
    )
    ident = np.eye(128, dtype=np.float32)

    shared = {
        "fw": fw, "bw": bw, "fhw": fhw, "bhw": bhw,
        "fbt": fbt, "bbt": bbt, "fhbt": fhbt, "bhbt": bhbt,
        "padf": padf, "padb": padb, "ident": ident,
    }
    in_maps = []
    for i in range(NCORES):
        m = dict(shared)
        m["x"] = np.ascontiguousarray(x[i * BLOC:(i + 1) * BLOC])
        in_maps.append(m)
    return in_maps


def _run(inputs, trace=False, tmpdir=None):
    nc = _get_program()
    in_maps = _make_in_maps(inputs)
    res = run_bass_kernel_spmd(
        nc, in_maps, core_ids=list(range(NCORES)), trace=trace, tmpdir=tmpdir
    )
    out = np.concatenate([r["out"] for r in res.results], axis=1)
    return out, res


def kernel(**inputs):
    trace = bool(int(os.environ.get("BASS_KERNEL_TRACE", "0")))
    out, _ = _run(inputs, trace=trace)
    return out



# revision 5
# speedup vs baseline: 1.3469x; 1.3469x over previous
"""Trainium2 Bass kernel for the Bengio03 Highway BiLM problem.

Math (see reference): L=3 layers, each with fwd/bwd chains. Per (layer, dir):
  padded = [front_pads(4), seq(512), back_pads(4)]          # [B, 520, H]
  pre[t] = sum_{k=0..4} padded[t + k + off] @ W[k*H:(k+1)*H]  (off=0 fwd, 4 bwd)
  x0 = relu(pre + b)
  2x highway: proj = x @ Ws[j] + bs[j]; nonlin,gate = split(proj)
              x = sigmoid(gate)*x + (1-sigmoid(gate))*relu(nonlin)
  out[l,:,:, 0:512] = f chain, [512:1024] = b chain

Implementation strategy (per core, data-parallel over batch: 4 seqs/core):
  - Activations feature-major in SBUF. Layer-0 conv + all highway matmuls in
    bf16 (same PE column rate as fp32r, but LDWEIGHTS gets FWL and halves DMA).
  - Layer-1/2 convs in fp8e4 DoubleRow (2 MACs/cycle): weights are packed as
    [128, 2, H] plane pairs (adjacent 128-feature chunks of the same tap);
    activations as [128, 2, 528] plane-pair tiles (stride 528 % 16 == 0).
    Per-matrix power-of-2 weight scales; descale folded into the ScalarE
    relu activation (scale operand).
  - Output written feature-major ([L, BLOC, 2H, S] bf16, contiguous DMA);
    the host transposes back. End-to-end rel err ~1.3e-2 (gate 2e-2).
"""

import os
import sys

sys.path.insert(0, "/opt/trn_rl_repo")

import numpy as np
import ml_dtypes

import concourse.bass as bass
import concourse.bacc as bacc
import concourse.tile as tile
from concourse import mybir
from concourse.bass_utils import run_bass_kernel_spmd

# Problem constants (hardcoded per spec).
L = 3
WIDTH = 4
H = 512
B = 32
S = 512
NHW = 2
CIN = (WIDTH + 1) * H  # 2560
NCORES = 8
BLOC = B // NCORES  # 4 sequences per core
HC = H // 128  # 4 hidden chunks of 128
SPAD = S + 2 * WIDTH  # 520
SPAD8 = 528  # fp8 plane stride (multiple of 16 bytes)
NT = S // 128  # 4 token tiles of 128
F32 = mybir.dt.float32
BF16 = mybir.dt.bfloat16
F8 = mybir.dt.float8e4
RELU = mybir.ActivationFunctionType.Relu
SIGM = mybir.ActivationFunctionType.Sigmoid
ADD = mybir.AluOpType.add
MAX = mybir.AluOpType.max
DR = mybir.MatmulPerfMode.DoubleRow

NP_BF16 = ml_dtypes.bfloat16
NP_F8 = ml_dtypes.float8_e4m3fn


def _build_program():
    nc = bacc.Bacc(
        "TRN2",
        target_bir_lowering=False,
        debug=False,
        enable_asserts=False,
        num_devices=1,
    )

    x_d = nc.dram_tensor("x", [BLOC, S, H], F32, kind="ExternalInput").ap()
    # conv layer-0 weights, bf16 [CIN, H] per dir
    cw0f_d = nc.dram_tensor("cw0f", [CIN, H], BF16, kind="ExternalInput").ap()
    cw0b_d = nc.dram_tensor("cw0b", [CIN, H], BF16, kind="ExternalInput").ap()
    # conv layer-1/2 weights fp8 DoubleRow packs: [li, rr, 128, 2*H]
    cw8f_d = nc.dram_tensor("cw8f", [2, 10, 128, 2 * H], F8, kind="ExternalInput").ap()
    cw8b_d = nc.dram_tensor("cw8b", [2, 10, 128, 2 * H], F8, kind="ExternalInput").ap()
    # highway weights bf16
    fhw_d = nc.dram_tensor("fhw", [L, NHW, H, 2 * H], BF16, kind="ExternalInput").ap()
    bhw_d = nc.dram_tensor("bhw", [L, NHW, H, 2 * H], BF16, kind="ExternalInput").ap()
    # biases (transposed to [128, chunks]) fp32
    fbt_d = nc.dram_tensor("fbt", [L, 128, HC], F32, kind="ExternalInput").ap()
    bbt_d = nc.dram_tensor("bbt", [L, 128, HC], F32, kind="ExternalInput").ap()
    fhbt_d = nc.dram_tensor("fhbt", [L, NHW, 128, 2 * HC], F32, kind="ExternalInput").ap()
    bhbt_d = nc.dram_tensor("bhbt", [L, NHW, 128, 2 * HC], F32, kind="ExternalInput").ap()
    # conv descale factors for fp8 layers: [128, 4] cols = (dir f/b) * 2 + (li)
    csc_d = nc.dram_tensor("csc", [128, 4], F32, kind="ExternalInput").ap()
    # pads: layer-0 bf16 [H, 8] (front||back); layers 1,2 fp8 [2, H, 8]
    pad0_d = nc.dram_tensor("pad0", [H, 8], BF16, kind="ExternalInput").ap()
    pad8_d = nc.dram_tensor("pad8", [2, H, 8], F8, kind="ExternalInput").ap()
    id_d = nc.dram_tensor("ident", [128, 128], F32, kind="ExternalInput").ap()
    # output feature-major: rows 0:H = f chain, H:2H = b chain
    out_d = nc.dram_tensor("out", [L, BLOC, 2 * H, S], BF16, kind="ExternalOutput").ap()

    with tile.TileContext(nc) as tc:
        with (
            tc.tile_pool(name="consts", bufs=1) as consts,
            tc.tile_pool(name="acts", bufs=16) as acts,
            tc.tile_pool(name="raws", bufs=12) as raws_pool,
            tc.tile_pool(name="a8", bufs=16) as a8pool,
            tc.tile_pool(name="cw0", bufs=5) as cw0pool,
            tc.tile_pool(name="cw8", bufs=20) as cw8pool,
            tc.tile_pool(name="hww", bufs=6) as hwwpool,
            tc.tile_pool(name="xmid", bufs=8) as xmid,
            tc.tile_pool(name="work", bufs=3) as work,
            tc.tile_pool(name="psum", bufs=2, space="PSUM") as psum,
        ):
            # ---- constants ----
            ident = consts.tile([128, 128], F32, name="identc", tag="ident", bufs=1)
            nc.sync.dma_start(ident[:], id_d[:, :])

            pads0 = {}
            for c in range(HC):
                pt = consts.tile([128, 8], BF16, name=f"p0_{c}", tag="pads0", bufs=HC)
                nc.sync.dma_start(pt[:], pad0_d[c * 128:(c + 1) * 128, :])
                pads0[c] = pt
            pads8 = {}
            for li in range(2):
                for c in range(HC):
                    pt = consts.tile([128, 8], F8, name=f"p8_{li}_{c}", tag="pads8", bufs=2 * HC)
                    nc.sync.dma_start(pt[:], pad8_d[li, c * 128:(c + 1) * 128, :])
                    pads8[(li, c)] = pt

            cbias = {}
            for dirc, src in (("f", fbt_d), ("b", bbt_d)):
                for l in range(L):
                    t = consts.tile([128, HC], F32, name=f"cb_{dirc}{l}", tag="cb", bufs=2 * L)
                    nc.sync.dma_start(t[:], src[l])
                    cbias[(dirc, l)] = t
            cscale = consts.tile([128, 4], F32, name="cscale", tag="csc", bufs=1)
            nc.sync.dma_start(cscale[:], csc_d[:, :])
            hbias = {}
            for dirc, src in (("f", fhbt_d), ("b", bhbt_d)):
                for l in range(L):
                    for j in range(NHW):
                        t = consts.tile(
                            [128, 2 * HC], F32, name=f"hb_{dirc}{l}_{j}", tag="hb",
                            bufs=2 * L * NHW,
                        )
                        nc.sync.dma_start(t[:], src[l, j])
                        hbias[(dirc, l, j)] = t

            # ---- weight loading ----
            loaded_cw0 = {}
            loaded_cw8 = {}
            loaded_hw = {}

            def ensure_cw0(dirc):
                # layer-0 conv weights: 5 tiles [128, 4, 512] bf16 (r = 4*ri + q)
                if dirc not in loaded_cw0:
                    src = cw0f_d if dirc == "f" else cw0b_d
                    tiles = []
                    for ri in range(5):
                        w = cw0pool.tile(
                            [128, 4, H], BF16, name=f"cw0_{dirc}_{ri}", tag="cw0", bufs=5
                        )
                        nc.sync.dma_start(
                            w[:],
                            src[ri * 512:(ri + 1) * 512, :].rearrange(
                                "(q p) h -> p q h", p=128
                            ),
                        )
                        tiles.append(w)
                    loaded_cw0[dirc] = tiles
                return loaded_cw0[dirc]

            def ensure_cw8(dirc, l):
                # fp8 DR conv weights for layer l in {1,2}: 10 tiles [128, 2, 512]
                li = l - 1
                if (dirc, li) not in loaded_cw8:
                    src = cw8f_d if dirc == "f" else cw8b_d
                    tiles = []
                    for rr in range(10):
                        w = cw8pool.tile(
                            [128, 2, H], F8, name=f"cw8_{dirc}{li}_{rr}", tag="cw8", bufs=20
                        )
                        nc.sync.dma_start(
                            w[:],
                            src[li, rr].rearrange("p (i h) -> p i h", i=2),
                        )
                        tiles.append(w)
                    loaded_cw8[(dirc, li)] = tiles
                return loaded_cw8[(dirc, li)]

            def ensure_hww(dirc, l):
                # highway weights: per j one tile [128, 4, 1024] bf16
                if (dirc, l) not in loaded_hw:
                    src = fhw_d if dirc == "f" else bhw_d
                    res = []
                    for j in range(NHW):
                        w = hwwpool.tile(
                            [128, 4, 2 * H], BF16, name=f"hw_{dirc}{l}_{j}", tag="hww", bufs=6
                        )
                        nc.scalar.dma_start(
                            w[:],
                            src[l, j].rearrange("(h p) c -> p h c", p=128),
                        )
                        res.append(w)
                    loaded_hw[(dirc, l)] = res
                return loaded_hw[(dirc, l)]

            # ---- input stage: load + transpose to feature-major bf16 ----
            ensure_cw0("f")
            xT = {}
            raws = {}
            for b in range(BLOC):
                for t4 in range(NT):
                    raw = raws_pool.tile([128, H], F32, name=f"inraw_{b}_{t4}", tag="raws", bufs=12)
                    nc.scalar.dma_start(raw[:], x_d[b, t4 * 128:(t4 + 1) * 128, :])
                    raws[(b, t4)] = raw
            for b in range(BLOC):
                for c in range(HC):
                    at = acts.tile([128, SPAD], BF16, name=f"xT_{c}_{b}", tag="acts", bufs=16)
                    tp = psum.tile([128, S], F32, name=f"tpi_{c}_{b}", tag="hpsum", bufs=4)
                    for t4 in range(NT):
                        nc.tensor.matmul(
                            tp[:, t4 * 128:(t4 + 1) * 128],
                            lhsT=raws[(b, t4)][:, c * 128:(c + 1) * 128],
                            rhs=ident[:],
                            is_transpose=True,
                        )
                    nc.scalar.copy(at[:, WIDTH:WIDTH + S], tp[:])
                    nc.gpsimd.tensor_copy(at[:, 0:WIDTH], pads0[c][:, 0:WIDTH])
                    nc.gpsimd.tensor_copy(at[:, WIDTH + S:SPAD], pads0[c][:, WIDTH:])
                    xT[(c, b)] = at

            # ---- stages ----
            def conv0_stage(dirc, pair, wtiles):
                # layer-0 conv from bf16 xT tiles
                off0 = 0 if dirc == "f" else WIDTH
                x0 = {}
                for b in pair:
                    for n in range(HC):
                        ps = psum.tile([128, S], F32, name=f"cps_{b}_{n}", tag="cpsum", bufs=4)
                        for r in range(20):
                            k, ci = divmod(r, HC)
                            ri, q = divmod(r, 4)
                            off = off0 + k
                            nc.tensor.matmul(
                                ps[:],
                                lhsT=wtiles[ri][:, q, n * 128:(n + 1) * 128],
                                rhs=xT[(ci, b)][:, off:off + S],
                                start=(r == 0),
                                stop=(r == 19),
                            )
                        xt = xmid.tile([128, S], BF16, name=f"x0_{b}_{n}", tag="x0", bufs=8)
                        nc.vector.tensor_scalar(
                            xt[:], ps[:], cbias[(dirc, 0)][:, n:n + 1], 0.0, ADD, MAX
                        )
                        x0[(n, b)] = xt
                return x0

            def conv8_stage(dirc, l, srcset, pair, wtiles):
                # fp8 DoubleRow conv for layers 1,2; srcset: fp8 pair tiles (cc, b)
                off0 = 0 if dirc == "f" else WIDTH
                li = l - 1
                sci = (0 if dirc == "f" else 2) + li
                sc = cscale[:, sci:sci + 1]
                x0 = {}
                for b in pair:
                    for n in range(HC):
                        ps = psum.tile([128, S], F32, name=f"cps_{b}_{n}", tag="cpsum", bufs=4)
                        for rr in range(10):
                            k, cc = divmod(rr, 2)
                            off = off0 + k
                            nc.tensor.matmul(
                                ps[:],
                                lhsT=wtiles[rr][:, :, n * 128:(n + 1) * 128],
                                rhs=srcset[(cc, b)][:, :, off:off + S],
                                start=(rr == 0),
                                stop=(rr == 9),
                                perf_mode=DR,
                            )
                        xt = xmid.tile([128, S], BF16, name=f"x0_{b}_{n}", tag="x0", bufs=8)
                        nc.scalar.activation(
                            xt[:], ps[:], RELU, bias=cbias[(dirc, l)][:, n:n + 1], scale=sc
                        )
                        x0[(n, b)] = xt
                return x0

            def hw_stage(dirc, l, j, srcset, pair, wt, final):
                hb = hbias[(dirc, l, j)]
                outs = {}
                for b in pair:
                    for c in range(HC):
                        pnl = psum.tile([128, S], F32, name=f"hnl_{b}_{c}", tag="hpsum", bufs=4)
                        for h in range(HC):
                            nc.tensor.matmul(
                                pnl[:],
                                lhsT=wt[:, h, c * 128:(c + 1) * 128],
                                rhs=srcset[(h, b)][:],
                                start=(h == 0),
                                stop=(h == HC - 1),
                            )
                        pgt = psum.tile([128, S], F32, name=f"hgt_{b}_{c}", tag="hpsum", bufs=4)
                        for h in range(HC):
                            nc.tensor.matmul(
                                pgt[:],
                                lhsT=wt[:, h, H + c * 128:H + (c + 1) * 128],
                                rhs=srcset[(h, b)][:],
                                start=(h == 0),
                                stop=(h == HC - 1),
                            )
                        r = work.tile([128, S], BF16, name=f"r_{b}_{c}", tag="r", bufs=3)
                        nc.scalar.activation(r[:], pnl[:], RELU, bias=hb[:, c:c + 1])
                        g = work.tile([128, S], BF16, name=f"g_{b}_{c}", tag="g", bufs=3)
                        nc.scalar.activation(g[:], pgt[:], SIGM, bias=hb[:, HC + c:HC + c + 1])
                        d = work.tile([128, S], BF16, name=f"d_{b}_{c}", tag="d", bufs=3)
                        nc.vector.tensor_sub(d[:], srcset[(c, b)][:], r[:])
                        nc.vector.tensor_mul(d[:], g[:], d[:])
                        if final:
                            o = work.tile([128, S], BF16, name=f"fin_{b}_{c}", tag="fin", bufs=6)
                            nc.vector.tensor_add(o[:], d[:], r[:])
                            emit_out(dirc, l, o, c, b)
                            if l + 1 < L:
                                cc, i = divmod(c, 2)
                                at8 = ensure_a8(l + 1, dirc, cc, b)
                                nc.vector.tensor_copy(at8[:, i, WIDTH:WIDTH + S], o[:])
                            outs[(c, b)] = o
                        else:
                            o = xmid.tile([128, S], BF16, name=f"x1_{b}_{c}", tag="x1", bufs=8)
                            nc.vector.tensor_add(o[:], d[:], r[:])
                            outs[(c, b)] = o
                return outs

            # fp8 activation pair tiles for the next layer's conv, pads pre-written
            a8tiles = {}

            def ensure_a8(l, dirc, cc, b):
                key = (l, dirc, cc, b)
                if key not in a8tiles:
                    li = l - 1
                    at8 = a8pool.tile(
                        [128, 2, SPAD8], F8, name=f"a8_{dirc}{l}_{cc}_{b}", tag="a8", bufs=16
                    )
                    for i in range(2):
                        c = 2 * cc + i
                        nc.gpsimd.tensor_copy(at8[:, i, 0:WIDTH], pads8[(li, c)][:, 0:WIDTH])
                        nc.gpsimd.tensor_copy(
                            at8[:, i, WIDTH + S:SPAD], pads8[(li, c)][:, WIDTH:]
                        )
                    a8tiles[key] = at8
                return a8tiles[key]

            def emit_out(dirc, l, o, c, b):
                doff = 0 if dirc == "f" else H
                h0 = doff + c * 128
                eng = nc.sync if (c + b) % 2 == 0 else nc.gpsimd
                eng.dma_start(out_d[l, b, h0:h0 + 128, :], o[:])

            # ---- main chain: f fully, then b (xT stays resident for b) ----
            PAIRS = [(0, 1), (2, 3)]
            for dirc in ("f", "b"):
                for l in range(L):
                    if l == 0:
                        cw = ensure_cw0(dirc)
                    else:
                        cw = ensure_cw8(dirc, l)
                    hw = ensure_hww(dirc, l)
                    for pair in PAIRS:
                        if l == 0:
                            x0 = conv0_stage(dirc, pair, cw)
                        else:
                            src8 = {
                                (cc, b): a8tiles[(l, dirc, cc, b)]
                                for cc in range(2)
                                for b in pair
                            }
                            x0 = conv8_stage(dirc, l, src8, pair, cw)
                        x1 = hw_stage(dirc, l, 0, x0, pair, hw[0], final=False)
                        hw_stage(dirc, l, 1, x1, pair, hw[1], final=True)

    nc.compile()
    return nc


_CACHE = {}


def _get_program():
    if "nc" not in _CACHE:
        _CACHE["nc"] = _build_program()
    return _CACHE["nc"]


def _q8(w, s):
    # quantize to TRN fp8e4 grid (values kept <= 120, identical to OCP e4m3fn)
    return np.asarray(np.clip(w * s, -240.0, 240.0), dtype=np.float32).astype(NP_F8)


def _pack_cw8(W):
    # W: [CIN, H] fp32 -> scale + [10, 128, 2H] fp8 DoubleRow pack
    # rr = k*2 + cc pairs row chunks r0 = k*HC + 2cc, r1 = r0 + 1
    s = 2.0 ** np.floor(np.log2(120.0 / np.abs(W).max()))
    Wq = _q8(W, s)
    out = np.zeros((10, 128, 2 * H), dtype=NP_F8)
    for rr in range(10):
        k, cc = divmod(rr, 2)
        r0 = k * HC + 2 * cc
        out[rr, :, 0:H] = Wq[r0 * 128:(r0 + 1) * 128, :]
        out[rr, :, H:] = Wq[(r0 + 1) * 128:(r0 + 2) * 128, :]
    return s, out


def _make_in_maps(inputs):
    x = np.ascontiguousarray(inputs["inputs"], dtype=np.float32)
    fw = np.asarray(inputs["fwd_W"], dtype=np.float32)
    bw = np.asarray(inputs["bwd_W"], dtype=np.float32)

    cw0f = np.ascontiguousarray(fw[0]).astype(NP_BF16)
    cw0b = np.ascontiguousarray(bw[0]).astype(NP_BF16)
    sf1, f1 = _pack_cw8(fw[1])
    sf2, f2 = _pack_cw8(fw[2])
    sb1, b1 = _pack_cw8(bw[1])
    sb2, b2 = _pack_cw8(bw[2])
    cw8f = np.stack([f1, f2], axis=0)
    cw8b = np.stack([b1, b2], axis=0)
    csc = np.empty((128, 4), dtype=np.float32)
    csc[:, 0] = 1.0 / sf1
    csc[:, 1] = 1.0 / sf2
    csc[:, 2] = 1.0 / sb1
    csc[:, 3] = 1.0 / sb2

    fhw = np.asarray(inputs["fwd_hw_W"], dtype=np.float32).astype(NP_BF16)
    bhw = np.asarray(inputs["bwd_hw_W"], dtype=np.float32).astype(NP_BF16)

    fbt = np.ascontiguousarray(
        np.asarray(inputs["fwd_b"], dtype=np.float32).reshape(L, HC, 128).transpose(0, 2, 1)
    )
    bbt = np.ascontiguousarray(
        np.asarray(inputs["bwd_b"], dtype=np.float32).reshape(L, HC, 128).transpose(0, 2, 1)
    )
    fhbt = np.ascontiguousarray(
        np.asarray(inputs["fwd_hw_b"], dtype=np.float32)
        .reshape(L, NHW, 2 * HC, 128)
        .transpose(0, 1, 3, 2)
    )
    bhbt = np.ascontiguousarray(
        np.asarray(inputs["bwd_hw_b"], dtype=np.float32)
        .reshape(L, NHW, 2 * HC, 128)
        .transpose(0, 1, 3, 2)
    )
    fp = np.asarray(inputs["fwd_pads"], dtype=np.float32)  # [L, 4, H]
    bp = np.asarray(inputs["bwd_pads"], dtype=np.float32)
    # layer-l pads: front = fwd_pads[l] (cols 0:4), back = bwd_pads[l] (cols 4:8)
    pad0 = np.concatenate([fp[0].T, bp[0].T], axis=1).astype(NP_BF16)  # [H, 8]
    pad8 = np.stack(
        [
            np.concatenate([fp[l].T, bp[l].T], axis=1).astype(NP_F8)
            for l in (1, 2)
        ],
        axis=0,
    )
    ident = np.eye(128, dtype=np.float32)

    shared = {
        "cw0f": cw0f, "cw0b": cw0b, "cw8f": cw8f, "cw8b": cw8b,
        "fhw": fhw, "bhw": bhw,
        "fbt": fbt, "bbt": bbt, "fhbt": fhbt, "bhbt": bhbt,
        "csc": csc, "pad0": pad0, "pad8": pad8, "ident": ident,
    }
    in_maps = []
    for i in range(NCORES):
        m = dict(shared)
        m["x"] = np.ascontiguousarray(x[i * BLOC:(i + 1) * BLOC])
        in_maps.append(m)
    return in_maps


def _run(inputs, trace=False, tmpdir=None):
    nc = _get_program()
    in_maps = _make_in_maps(inputs)
    res = run_bass_kernel_spmd(
        nc, in_maps, core_ids=list(range(NCORES)), trace=trace, tmpdir=tmpdir
    )
    # out: [L, BLOC, 2H, S] bf16 feature-major -> [L, B, S, 2H] fp32
    parts = [
        np.asarray(r["out"]).astype(np.float32).transpose(0, 1, 3, 2)
        for r in res.results
    ]
    out = np.concatenate(parts, axis=1)
    return np.ascontiguousarray(out), res


def kernel(**inputs):
    trace = bool(int(os.environ.get("BASS_KERNEL_TRACE", "0")))
    out, _ = _run(inputs, trace=trace)
    return out


# revision 10
# speedup vs baseline: 1.3685x; 1.0160x over previous
"""Trainium2 Bass kernel for the Bengio03 Highway BiLM problem.

Math (see reference): L=3 layers, each with fwd/bwd chains. Per (layer, dir):
  padded = [front_pads(4), seq(512), back_pads(4)]          # [B, 520, H]
  pre[t] = sum_{k=0..4} padded[t + k + off] @ W[k*H:(k+1)*H]  (off=0 fwd, 4 bwd)
  x0 = relu(pre + b)
  2x highway: proj = x @ Ws[j] + bs[j]; nonlin,gate = split(proj)
              x = sigmoid(gate)*x + (1-sigmoid(gate))*relu(nonlin)
  out[l,:,:, 0:512] = f chain, [512:1024] = b chain

Implementation strategy (per core, data-parallel over batch: 4 seqs/core):
  - Activations feature-major in SBUF. Layer-0 conv + highway matmuls in
    bf16 (same PE column rate as fp32r, but LDWEIGHTS gets FWL and halves DMA).
  - Layer-1/2 convs (and optionally layer-2 highways) in fp8e4 DoubleRow
    (2 MACs/cycle): weights packed as [128, 2, H] plane pairs (adjacent
    128-feature chunks of the same tap); activations as [128, 2, 528]
    plane-pair tiles (stride 528 % 16 == 0). Per-matrix power-of-2 weight
    scales; descale folded into the ScalarE activation (scale operand).
  - Output written feature-major ([L, BLOC, 2H, S] bf16, contiguous DMA);
    the host transposes back. End-to-end rel err ~1.3-1.7e-2 (gate 2e-2).
"""

import os
import sys

sys.path.insert(0, "/opt/trn_rl_repo")

import numpy as np
import ml_dtypes

import concourse.bass as bass
import concourse.bacc as bacc
import concourse.tile as tile
from concourse import mybir
from concourse.bass_utils import run_bass_kernel_spmd

# Problem constants (hardcoded per spec).
L = 3
WIDTH = 4
H = 512
B = 32
S = 512
NHW = 2
CIN = (WIDTH + 1) * H  # 2560
NCORES = 8
BLOC = B // NCORES  # 4 sequences per core
HC = H // 128  # 4 hidden chunks of 128
SPAD = S + 2 * WIDTH  # 520
SPAD8 = 528  # fp8 plane stride (multiple of 16 bytes)
NT = S // 128  # 4 token tiles of 128
F32 = mybir.dt.float32
BF16 = mybir.dt.bfloat16
F8 = mybir.dt.float8e4
RELU = mybir.ActivationFunctionType.Relu
SIGM = mybir.ActivationFunctionType.Sigmoid
ADD = mybir.AluOpType.add
MAX = mybir.AluOpType.max
DR = mybir.MatmulPerfMode.DoubleRow

# layer-2 highway in fp8 DoubleRow as well (rel err ~1.65e-2 vs 1.33e-2)
HW2_FP8 = True

NP_BF16 = ml_dtypes.bfloat16
NP_F8 = ml_dtypes.float8_e4m3fn


def _build_program():
    nc = bacc.Bacc(
        "TRN2",
        target_bir_lowering=False,
        debug=False,
        enable_asserts=False,
        num_devices=1,
    )

    x_d = nc.dram_tensor("x", [BLOC, S, H], F32, kind="ExternalInput").ap()
    # conv layer-0 weights, bf16 [CIN, H] per dir
    cw0f_d = nc.dram_tensor("cw0f", [CIN, H], BF16, kind="ExternalInput").ap()
    cw0b_d = nc.dram_tensor("cw0b", [CIN, H], BF16, kind="ExternalInput").ap()
    # conv layer-1/2 weights fp8 DoubleRow packs: [li, rr, 128, 2*H]
    cw8f_d = nc.dram_tensor("cw8f", [2, 10, 128, 2 * H], F8, kind="ExternalInput").ap()
    cw8b_d = nc.dram_tensor("cw8b", [2, 10, 128, 2 * H], F8, kind="ExternalInput").ap()
    # highway weights bf16 (layers 0..HWL_BF-1)
    fhw_d = nc.dram_tensor("fhw", [L, NHW, H, 2 * H], BF16, kind="ExternalInput").ap()
    bhw_d = nc.dram_tensor("bhw", [L, NHW, H, 2 * H], BF16, kind="ExternalInput").ap()
    # layer-2 highway weights fp8 DR: [j, half(nl/gt), hh, 128, 2*H]
    fhw8_d = nc.dram_tensor("fhw8", [NHW, 2, 2, 128, 2 * H], F8, kind="ExternalInput").ap()
    bhw8_d = nc.dram_tensor("bhw8", [NHW, 2, 2, 128, 2 * H], F8, kind="ExternalInput").ap()
    # biases (transposed to [128, chunks]) fp32
    fbt_d = nc.dram_tensor("fbt", [L, 128, HC], F32, kind="ExternalInput").ap()
    bbt_d = nc.dram_tensor("bbt", [L, 128, HC], F32, kind="ExternalInput").ap()
    fhbt_d = nc.dram_tensor("fhbt", [L, NHW, 128, 2 * HC], F32, kind="ExternalInput").ap()
    bhbt_d = nc.dram_tensor("bhbt", [L, NHW, 128, 2 * HC], F32, kind="ExternalInput").ap()
    # descale factors: cols 0-3 conv (dir*2+li); cols 4-11 hw l2 (dir*4+j*2+half)
    csc_d = nc.dram_tensor("csc", [128, 12], F32, kind="ExternalInput").ap()
    # pads: layer-0 bf16 [H, 8] (front||back); layers 1,2 fp8 [2, H, 8]
    pad0_d = nc.dram_tensor("pad0", [H, 8], BF16, kind="ExternalInput").ap()
    pad8_d = nc.dram_tensor("pad8", [2, H, 8], F8, kind="ExternalInput").ap()
    id_d = nc.dram_tensor("ident", [128, 128], F32, kind="ExternalInput").ap()
    # output feature-major: rows 0:H = f chain, H:2H = b chain
    out_d = nc.dram_tensor("out", [L, BLOC, 2 * H, S], BF16, kind="ExternalOutput").ap()

    with tile.TileContext(nc) as tc:
        with (
            tc.tile_pool(name="consts", bufs=1) as consts,
            tc.tile_pool(name="acts", bufs=16) as acts,
            tc.tile_pool(name="raws", bufs=8) as raws_pool,
            tc.tile_pool(name="a8", bufs=16) as a8pool,
            tc.tile_pool(name="h8", bufs=12) as h8pool,
            tc.tile_pool(name="cw0", bufs=5) as cw0pool,
            tc.tile_pool(name="cw8", bufs=20) as cw8pool,
            tc.tile_pool(name="hww", bufs=4) as hwwpool,
            tc.tile_pool(name="hw8", bufs=8) as hw8pool,
            tc.tile_pool(name="xmid", bufs=8) as xmid,
            tc.tile_pool(name="work", bufs=3) as work,
            tc.tile_pool(name="psum", bufs=2, space="PSUM") as psum,
        ):
            # ---- hot-path constants / weights first (parallel queues) ----
            ident = consts.tile([128, 128], F32, name="identc", tag="ident", bufs=1)
            nc.sync.dma_start(ident[:], id_d[:, :])

            pads0 = {}
            for c in range(HC):
                pt = consts.tile([128, 8], BF16, name=f"p0_{c}", tag="pads0", bufs=HC)
                nc.scalar.dma_start(pt[:], pad0_d[c * 128:(c + 1) * 128, :])
                pads0[c] = pt

            loaded_cw0 = {}
            loaded_cw8 = {}
            loaded_hw = {}
            loaded_hw8 = {}

            def ensure_cw0(dirc):
                # layer-0 conv weights: 5 tiles [128, 4, 512] bf16 (r = 4*ri + q)
                if dirc not in loaded_cw0:
                    src = cw0f_d if dirc == "f" else cw0b_d
                    tiles = []
                    for ri in range(5):
                        w = cw0pool.tile(
                            [128, 4, H], BF16, name=f"cw0_{dirc}_{ri}", tag="cw0", bufs=5
                        )
                        eng = nc.sync if ri % 2 == 0 else nc.gpsimd
                        eng.dma_start(
                            w[:],
                            src[ri * 512:(ri + 1) * 512, :].rearrange(
                                "(q p) h -> p q h", p=128
                            ),
                        )
                        tiles.append(w)
                    loaded_cw0[dirc] = tiles
                return loaded_cw0[dirc]

            def ensure_cw8(dirc, l):
                # fp8 DR conv weights for layer l in {1,2}: 10 tiles [128, 2, 512]
                li = l - 1
                if (dirc, li) not in loaded_cw8:
                    src = cw8f_d if dirc == "f" else cw8b_d
                    tiles = []
                    for rr in range(10):
                        w = cw8pool.tile(
                            [128, 2, H], F8, name=f"cw8_{dirc}{li}_{rr}", tag="cw8", bufs=20
                        )
                        nc.sync.dma_start(
                            w[:],
                            src[li, rr].rearrange("p (i h) -> p i h", i=2),
                        )
                        tiles.append(w)
                    loaded_cw8[(dirc, li)] = tiles
                return loaded_cw8[(dirc, li)]

            def ensure_hww(dirc, l):
                # bf16 highway weights: per j one tile [128, 4, 1024]
                if (dirc, l) not in loaded_hw:
                    src = fhw_d if dirc == "f" else bhw_d
                    res = []
                    for j in range(NHW):
                        w = hwwpool.tile(
                            [128, 4, 2 * H], BF16, name=f"hw_{dirc}{l}_{j}", tag="hww", bufs=4
                        )
                        nc.scalar.dma_start(
                            w[:],
                            src[l, j].rearrange("(h p) c -> p h c", p=128),
                        )
                        res.append(w)
                    loaded_hw[(dirc, l)] = res
                return loaded_hw[(dirc, l)]

            def ensure_hw8(dirc):
                # fp8 DR layer-2 highway weights: tiles [(j, half, hh)] [128, 2, H]
                if dirc not in loaded_hw8:
                    src = fhw8_d if dirc == "f" else bhw8_d
                    res = {}
                    for j in range(NHW):
                        for half in range(2):
                            for hh in range(2):
                                w = hw8pool.tile(
                                    [128, 2, H], F8,
                                    name=f"hw8_{dirc}{j}_{half}_{hh}", tag="hw8", bufs=8,
                                )
                                nc.gpsimd.dma_start(
                                    w[:],
                                    src[j, half, hh].rearrange("p (i h) -> p i h", i=2),
                                )
                                res[(j, half, hh)] = w
                    loaded_hw8[dirc] = res
                return loaded_hw8[dirc]

            # ---- input stage: load + transpose to feature-major bf16 ----
            xT = {}
            raws = {}
            for b in (1, 0, 2, 3):
                for t4 in range(NT):
                    raw = raws_pool.tile([128, H], F32, name=f"inraw_{b}_{t4}", tag="raws", bufs=8)
                    eng = nc.gpsimd if b == 1 else nc.scalar
                    eng.dma_start(raw[:], x_d[b, t4 * 128:(t4 + 1) * 128, :])
                    raws[(b, t4)] = raw
            ensure_cw0("f")
            for b in range(BLOC):
                for c in range(HC):
                    at = acts.tile([128, SPAD], BF16, name=f"xT_{c}_{b}", tag="acts", bufs=16)
                    tp = psum.tile([128, S], F32, name=f"tpi_{c}_{b}", tag="hpsum", bufs=4)
                    for t4 in range(NT):
                        nc.tensor.matmul(
                            tp[:, t4 * 128:(t4 + 1) * 128],
                            lhsT=raws[(b, t4)][:, c * 128:(c + 1) * 128],
                            rhs=ident[:],
                            is_transpose=True,
                        )
                    nc.scalar.copy(at[:, WIDTH:WIDTH + S], tp[:])
                    nc.gpsimd.tensor_copy(at[:, 0:WIDTH], pads0[c][:, 0:WIDTH])
                    nc.gpsimd.tensor_copy(at[:, WIDTH + S:SPAD], pads0[c][:, WIDTH:])
                    xT[(c, b)] = at

            # ---- remaining constants (cold path) ----
            pads8 = {}
            for li in range(2):
                for c in range(HC):
                    pt = consts.tile([128, 8], F8, name=f"p8_{li}_{c}", tag="pads8", bufs=2 * HC)
                    nc.sync.dma_start(pt[:], pad8_d[li, c * 128:(c + 1) * 128, :])
                    pads8[(li, c)] = pt
            cbias = {}
            for dirc, src in (("f", fbt_d), ("b", bbt_d)):
                for l in range(L):
                    t = consts.tile([128, HC], F32, name=f"cb_{dirc}{l}", tag="cb", bufs=2 * L)
                    nc.sync.dma_start(t[:], src[l])
                    cbias[(dirc, l)] = t
            cscale = consts.tile([128, 12], F32, name="cscale", tag="csc", bufs=1)
            nc.sync.dma_start(cscale[:], csc_d[:, :])
            hbias = {}
            for dirc, src in (("f", fhbt_d), ("b", bhbt_d)):
                for l in range(L):
                    for j in range(NHW):
                        t = consts.tile(
                            [128, 2 * HC], F32, name=f"hb_{dirc}{l}_{j}", tag="hb",
                            bufs=2 * L * NHW,
                        )
                        nc.sync.dma_start(t[:], src[l, j])
                        hbias[(dirc, l, j)] = t

            # ---- stages ----
            def conv0_stage(dirc, pair, wtiles):
                # layer-0 conv from bf16 xT tiles
                off0 = 0 if dirc == "f" else WIDTH
                x0 = {}
                for b in pair:
                    for n in range(HC):
                        ps = psum.tile([128, S], F32, name=f"cps_{b}_{n}", tag="cpsum", bufs=4)
                        for r in range(20):
                            k, ci = divmod(r, HC)
                            ri, q = divmod(r, 4)
                            off = off0 + k
                            nc.tensor.matmul(
                                ps[:],
                                lhsT=wtiles[ri][:, q, n * 128:(n + 1) * 128],
                                rhs=xT[(ci, b)][:, off:off + S],
                                start=(r == 0),
                                stop=(r == 19),
                            )
                        xt = xmid.tile([128, S], BF16, name=f"x0_{b}_{n}", tag="x0", bufs=8)
                        nc.vector.tensor_scalar(
                            xt[:], ps[:], cbias[(dirc, 0)][:, n:n + 1], 0.0, ADD, MAX
                        )
                        x0[(n, b)] = xt
                return x0

            def conv8_stage(dirc, l, srcset, pair, wtiles):
                # fp8 DoubleRow conv for layers 1,2; srcset: fp8 pair tiles (cc, b)
                off0 = 0 if dirc == "f" else WIDTH
                li = l - 1
                sci = (0 if dirc == "f" else 2) + li
                sc = cscale[:, sci:sci + 1]
                hw8 = HW2_FP8 and l == 2
                x0 = {}
                x0f8 = {}
                for b in pair:
                    for n in range(HC):
                        ps = psum.tile([128, S], F32, name=f"cps_{b}_{n}", tag="cpsum", bufs=4)
                        for rr in range(10):
                            k, cc = divmod(rr, 2)
                            off = off0 + k
                            nc.tensor.matmul(
                                ps[:],
                                lhsT=wtiles[rr][:, :, n * 128:(n + 1) * 128],
                                rhs=srcset[(cc, b)][:, :, off:off + S],
                                start=(rr == 0),
                                stop=(rr == 9),
                                perf_mode=DR,
                            )
                        xt = xmid.tile([128, S], BF16, name=f"x0_{b}_{n}", tag="x0", bufs=8)
                        nc.scalar.activation(
                            xt[:], ps[:], RELU, bias=cbias[(dirc, l)][:, n:n + 1], scale=sc
                        )
                        x0[(n, b)] = xt
                        if hw8:
                            hh, i = divmod(n, 2)
                            key = (hh, b)
                            if key not in x0f8:
                                x0f8[key] = h8pool.tile(
                                    [128, 2, S], F8, name=f"x08_{b}_{hh}", tag="h8", bufs=12
                                )
                            nc.vector.tensor_copy(x0f8[key][:, i, :], xt[:])
                return (x0, x0f8) if hw8 else x0

            def hw_stage(dirc, l, j, srcset, pair, wt, final):
                hb = hbias[(dirc, l, j)]
                outs = {}
                for b in pair:
                    for c in range(HC):
                        pnl = psum.tile([128, S], F32, name=f"hnl_{b}_{c}", tag="hpsum", bufs=4)
                        for h in range(HC):
                            nc.tensor.matmul(
                                pnl[:],
                                lhsT=wt[:, h, c * 128:(c + 1) * 128],
                                rhs=srcset[(h, b)][:],
                                start=(h == 0),
                                stop=(h == HC - 1),
                            )
                        pgt = psum.tile([128, S], F32, name=f"hgt_{b}_{c}", tag="hpsum", bufs=4)
                        for h in range(HC):
                            nc.tensor.matmul(
                                pgt[:],
                                lhsT=wt[:, h, H + c * 128:H + (c + 1) * 128],
                                rhs=srcset[(h, b)][:],
                                start=(h == 0),
                                stop=(h == HC - 1),
                            )
                        finish_hw(dirc, l, j, b, c, pnl, pgt, srcset, outs, final,
                                  hb[:, c:c + 1], hb[:, HC + c:HC + c + 1], None, None)
                return outs

            def hw8_stage(dirc, j, srcbf, src8, pair, w8, final):
                # layer-2 highway with fp8 DR matmuls
                l = 2
                hb = hbias[(dirc, l, j)]
                sbase = 4 + (0 if dirc == "f" else 4) + j * 2
                sc_nl = cscale[:, sbase:sbase + 1]
                sc_gt = cscale[:, sbase + 1:sbase + 2]
                outs = {}
                for b in pair:
                    for c in range(HC):
                        pnl = psum.tile([128, S], F32, name=f"hnl_{b}_{c}", tag="hpsum", bufs=4)
                        for hh in range(2):
                            nc.tensor.matmul(
                                pnl[:],
                                lhsT=w8[(j, 0, hh)][:, :, c * 128:(c + 1) * 128],
                                rhs=src8[(hh, b)][:],
                                start=(hh == 0),
                                stop=(hh == 1),
                                perf_mode=DR,
                            )
                        pgt = psum.tile([128, S], F32, name=f"hgt_{b}_{c}", tag="hpsum", bufs=4)
                        for hh in range(2):
                            nc.tensor.matmul(
                                pgt[:],
                                lhsT=w8[(j, 1, hh)][:, :, c * 128:(c + 1) * 128],
                                rhs=src8[(hh, b)][:],
                                start=(hh == 0),
                                stop=(hh == 1),
                                perf_mode=DR,
                            )
                        finish_hw(dirc, l, j, b, c, pnl, pgt, srcbf, outs, final,
                                  hb[:, c:c + 1], hb[:, HC + c:HC + c + 1], sc_nl, sc_gt)
                return outs

            def finish_hw(dirc, l, j, b, c, pnl, pgt, srcset, outs, final,
                          bnl, bgt, sc_nl, sc_gt):
                hw8next = HW2_FP8 and l == 2 and j == 0
                r = work.tile([128, S], BF16, name=f"r_{b}_{c}", tag="r", bufs=3)
                if sc_nl is None:
                    nc.scalar.activation(r[:], pnl[:], RELU, bias=bnl)
                else:
                    nc.scalar.activation(r[:], pnl[:], RELU, bias=bnl, scale=sc_nl)
                g = work.tile([128, S], BF16, name=f"g_{b}_{c}", tag="g", bufs=3)
                if sc_gt is None:
                    nc.scalar.activation(g[:], pgt[:], SIGM, bias=bgt)
                else:
                    nc.scalar.activation(g[:], pgt[:], SIGM, bias=bgt, scale=sc_gt)
                d = work.tile([128, S], BF16, name=f"d_{b}_{c}", tag="d", bufs=3)
                nc.vector.tensor_sub(d[:], srcset[(c, b)][:], r[:])
                nc.vector.tensor_mul(d[:], g[:], d[:])
                if final:
                    o = work.tile([128, S], BF16, name=f"fin_{b}_{c}", tag="fin", bufs=4)
                    nc.vector.tensor_add(o[:], d[:], r[:])
                    emit_out(dirc, l, o, c, b)
                    if l + 1 < L:
                        cc, i = divmod(c, 2)
                        at8 = ensure_a8(l + 1, dirc, cc, b)
                        nc.vector.tensor_copy(at8[:, i, WIDTH:WIDTH + S], o[:])
                    outs[(c, b)] = o
                else:
                    o = xmid.tile([128, S], BF16, name=f"x1_{b}_{c}", tag="x1", bufs=8)
                    nc.vector.tensor_add(o[:], d[:], r[:])
                    outs[(c, b)] = o
                    if hw8next:
                        hh, i = divmod(c, 2)
                        key = ("x18", l, dirc, hh, b)
                        if key not in x18tiles:
                            x18tiles[key] = h8pool.tile(
                                [128, 2, S], F8, name=f"x18_{b}_{hh}", tag="h8", bufs=12
                            )
                        nc.vector.tensor_copy(x18tiles[key][:, i, :], o[:])

            x18tiles = {}

            # fp8 activation pair tiles for the next layer's conv, pads pre-written
            a8tiles = {}

            def ensure_a8(l, dirc, cc, b):
                key = (l, dirc, cc, b)
                if key not in a8tiles:
                    li = l - 1
                    at8 = a8pool.tile(
                        [128, 2, SPAD8], F8, name=f"a8_{dirc}{l}_{cc}_{b}", tag="a8", bufs=16
                    )
                    for i in range(2):
                        c = 2 * cc + i
                        nc.gpsimd.tensor_copy(at8[:, i, 0:WIDTH], pads8[(li, c)][:, 0:WIDTH])
                        nc.gpsimd.tensor_copy(
                            at8[:, i, WIDTH + S:SPAD], pads8[(li, c)][:, WIDTH:]
                        )
                    a8tiles[key] = at8
                return a8tiles[key]

            def emit_out(dirc, l, o, c, b):
                doff = 0 if dirc == "f" else H
                h0 = doff + c * 128
                eng = nc.sync if (c + b) % 2 == 0 else nc.gpsimd
                eng.dma_start(out_d[l, b, h0:h0 + 128, :], o[:])

            # ---- main chain: f fully, then b (xT stays resident for b) ----
            PAIRS = [(0, 1), (2, 3)]
            for dirc in ("f", "b"):
                for l in range(L):
                    use_hw8 = HW2_FP8 and l == 2
                    if l == 0:
                        cw = ensure_cw0(dirc)
                    else:
                        cw = ensure_cw8(dirc, l)
                    if use_hw8:
                        w8 = ensure_hw8(dirc)
                    else:
                        hw = ensure_hww(dirc, l)
                    for pair in PAIRS:
                        if l == 0:
                            x0 = conv0_stage(dirc, pair, cw)
                        else:
                            src8 = {
                                (cc, b): a8tiles[(l, dirc, cc, b)]
                                for cc in range(2)
                                for b in pair
                            }
                            res = conv8_stage(dirc, l, src8, pair, cw)
                            if use_hw8:
                                x0, x0f8 = res
                            else:
                                x0 = res
                        if use_hw8:
                            x1 = hw8_stage(dirc, 0, x0, x0f8, pair, w8, final=False)
                            x18 = {
                                (hh, b): x18tiles[("x18", 2, dirc, hh, b)]
                                for hh in range(2)
                                for b in pair
                            }
                            hw8_stage(dirc, 1, x1, x18, pair, w8, final=True)
                        else:
                            x1 = hw_stage(dirc, l, 0, x0, pair, hw[0], final=False)
                            hw_stage(dirc, l, 1, x1, pair, hw[1], final=True)

    nc.compile()
    return nc


_CACHE = {}


def _get_program():
    if "nc" not in _CACHE:
        _CACHE["nc"] = _build_program()
    return _CACHE["nc"]


def _q8(w, s):
    # quantize to TRN fp8e4 grid (values kept <= 120, identical to OCP e4m3fn)
    return np.asarray(np.clip(w * s, -240.0, 240.0), dtype=np.float32).astype(NP_F8)


def _p2scale(w):
    return 2.0 ** np.floor(np.log2(120.0 / np.abs(w).max()))


def _pack_cw8(W):
    # W: [CIN, H] fp32 -> scale + [10, 128, 2H] fp8 DoubleRow pack
    # rr = k*2 + cc pairs row chunks r0 = k*HC + 2cc, r1 = r0 + 1
    s = _p2scale(W)
    Wq = _q8(W, s)
    out = np.zeros((10, 128, 2 * H), dtype=NP_F8)
    for rr in range(10):
        k, cc = divmod(rr, 2)
        r0 = k * HC + 2 * cc
        out[rr, :, 0:H] = Wq[r0 * 128:(r0 + 1) * 128, :]
        out[rr, :, H:] = Wq[(r0 + 1) * 128:(r0 + 2) * 128, :]
    return s, out


def _pack_hw8(W):
    # W: [NHW, H, 2H] fp32 -> scales [NHW, 2], pack [NHW, 2, 2, 128, 2H] fp8
    scales = np.zeros((NHW, 2), dtype=np.float64)
    out = np.zeros((NHW, 2, 2, 128, 2 * H), dtype=NP_F8)
    for j in range(NHW):
        for half in range(2):
            blk = W[j][:, half * H:(half + 1) * H]  # [H, H]
            s = _p2scale(blk)
            scales[j, half] = s
            q = _q8(blk, s)
            for hh in range(2):
                out[j, half, hh, :, 0:H] = q[(2 * hh) * 128:(2 * hh + 1) * 128, :]
                out[j, half, hh, :, H:] = q[(2 * hh + 1) * 128:(2 * hh + 2) * 128, :]
    return scales, out


def _make_in_maps(inputs):
    x = np.ascontiguousarray(inputs["inputs"], dtype=np.float32)
    fw = np.asarray(inputs["fwd_W"], dtype=np.float32)
    bw = np.asarray(inputs["bwd_W"], dtype=np.float32)

    cw0f = np.ascontiguousarray(fw[0]).astype(NP_BF16)
    cw0b = np.ascontiguousarray(bw[0]).astype(NP_BF16)
    sf1, f1 = _pack_cw8(fw[1])
    sf2, f2 = _pack_cw8(fw[2])
    sb1, b1 = _pack_cw8(bw[1])
    sb2, b2 = _pack_cw8(bw[2])
    cw8f = np.stack([f1, f2], axis=0)
    cw8b = np.stack([b1, b2], axis=0)

    fhwf = np.asarray(inputs["fwd_hw_W"], dtype=np.float32)
    bhwf = np.asarray(inputs["bwd_hw_W"], dtype=np.float32)
    fhw = fhwf.astype(NP_BF16)
    bhw = bhwf.astype(NP_BF16)
    fsc, fhw8 = _pack_hw8(fhwf[2])
    bsc, bhw8 = _pack_hw8(bhwf[2])

    csc = np.empty((128, 12), dtype=np.float32)
    csc[:, 0] = 1.0 / sf1
    csc[:, 1] = 1.0 / sf2
    csc[:, 2] = 1.0 / sb1
    csc[:, 3] = 1.0 / sb2
    for j in range(NHW):
        for half in range(2):
            csc[:, 4 + j * 2 + half] = 1.0 / fsc[j, half]
            csc[:, 8 + j * 2 + half] = 1.0 / bsc[j, half]

    fbt = np.ascontiguousarray(
        np.asarray(inputs["fwd_b"], dtype=np.float32).reshape(L, HC, 128).transpose(0, 2, 1)
    )
    bbt = np.ascontiguousarray(
        np.asarray(inputs["bwd_b"], dtype=np.float32).reshape(L, HC, 128).transpose(0, 2, 1)
    )
    fhbt = np.ascontiguousarray(
        np.asarray(inputs["fwd_hw_b"], dtype=np.float32)
        .reshape(L, NHW, 2 * HC, 128)
        .transpose(0, 1, 3, 2)
    )
    bhbt = np.ascontiguousarray(
        np.asarray(inputs["bwd_hw_b"], dtype=np.float32)
        .reshape(L, NHW, 2 * HC, 128)
        .transpose(0, 1, 3, 2)
    )
    fp = np.asarray(inputs["fwd_pads"], dtype=np.float32)  # [L, 4, H]
    bp = np.asarray(inputs["bwd_pads"], dtype=np.float32)
    # layer-l pads: front = fwd_pads[l] (cols 0:4), back = bwd_pads[l] (cols 4:8)
    pad0 = np.concatenate([fp[0].T, bp[0].T], axis=1).astype(NP_BF16)  # [H, 8]
    pad8 = np.stack(
        [
            np.concatenate([fp[l].T, bp[l].T], axis=1).astype(NP_F8)
            for l in (1, 2)
        ],
        axis=0,
    )
    ident = np.eye(128, dtype=np.float32)

    shared = {
        "cw0f": cw0f, "cw0b": cw0b, "cw8f": cw8f, "cw8b": cw8b,
        "fhw": fhw, "bhw": bhw, "fhw8": fhw8, "bhw8": bhw8,
        "fbt": fbt, "bbt": bbt, "fhbt": fhbt, "bhbt": bhbt,
        "csc": csc, "pad0": pad0, "pad8": pad8, "ident": ident,
    }
    in_maps = []
    for i in range(NCORES):
        m = dict(shared)
        m["x"] = np.ascontiguousarray(x[i * BLOC:(i + 1) * BLOC])
        in_maps.append(m)
    return in_maps


def _run(inputs, trace=False, tmpdir=None):
    nc = _get_program()
    in_maps = _make_in_maps(inputs)
    res = run_bass_kernel_spmd(
        nc, in_maps, core_ids=list(range(NCORES)), trace=trace, tmpdir=tmpdir
    )
    # out: [L, BLOC, 2H, S] bf16 feature-major -> [L, B, S, 2H] fp32
    parts = [
        np.asarray(r["out"]).astype(np.float32).transpose(0, 1, 3, 2)
        for r in res.results
    ]
    out = np.concatenate(parts, axis=1)
    return np.ascontiguousarray(out), res


def kernel(**inputs):
    trace = bool(int(os.environ.get("BASS_KERNEL_TRACE", "0")))
    out, _ = _run(inputs, trace=trace)
    return out
